# revision 19
# baseline (speedup 1.0000x reference)
"""AttnBlock (GroupNorm -> QKV -> full attention -> proj + residual) on 8
Trainium2 NeuronCores, data-parallel over the batch dimension (b=8, one
sample per core).

Layouts per core (sample):
  x:  (c=512, w=2048) fp32, channel tiles of 128 partitions.
  h:  GroupNorm(x) in f32r (feeds all matmuls).
  q,k: (c, w) f32r; vT: (w, c) f32r computed directly transposed.
  scores_i = q[:, i-tile].T @ k  -> PSUM (128, 2048); softmax without
  max-subtraction (scores are O(1) for this problem); exp fused with
  row-sum via ACT accum_out; normalization fused into the PE transpose by
  using diag(1/rowsum) as the transpose identity.
  ho = sum_j vT.T @ attT, proj with host-transposed wp, bias folded with
  wp @ bv, residual added from a streamed x slice.
"""

import functools

import numpy as np

B = 8
C = 512
W = 2048
G = 32
EPS = 1e-6
P = 128
CT = C // P          # 4 channel tiles
NW = W // 512        # 4 w-chunks of 512
IT = W // P          # 16 i-tiles
IGRP = 4             # i-tiles per ho/proj group
NG = IT // IGRP      # 4 groups

TRACE = False
DEBUG = False
LAST_EXEC_NS = None
LAST_TRACE_PATH = None


def _build_nc():
    import concourse.bass as bass
    import concourse.mybir as mybir
    import concourse.tile as tile
    from concourse import bacc
    from concourse.masks import make_identity

    f32 = mybir.dt.float32
    f32r = mybir.dt.float32r
    Ident = mybir.ActivationFunctionType.Identity
    Exp = mybir.ActivationFunctionType.Exp
    Sqrt = mybir.ActivationFunctionType.Sqrt
    mult = mybir.AluOpType.mult
    add = mybir.AluOpType.add
    subtract = mybir.AluOpType.subtract

    nc = bacc.Bacc()

    x_d = nc.declare_dram_parameter("x", [C, W], f32, isOutput=False)
    wqT_d = nc.declare_dram_parameter("wqT", [C, C], f32, isOutput=False)
    wkT_d = nc.declare_dram_parameter("wkT", [C, C], f32, isOutput=False)
    wvT_d = nc.declare_dram_parameter("wvT", [C, C], f32, isOutput=False)
    wpT_d = nc.declare_dram_parameter("wpT", [C, C], f32, isOutput=False)
    bq_d = nc.declare_dram_parameter("bq", [C, 1], f32, isOutput=False)
    bk_d = nc.declare_dram_parameter("bk", [C, 1], f32, isOutput=False)
    bp_d = nc.declare_dram_parameter("bp", [C, 1], f32, isOutput=False)
    gam_d = nc.declare_dram_parameter("gam", [C, 1], f32, isOutput=False)
    bet_d = nc.declare_dram_parameter("bet", [C, 1], f32, isOutput=False)
    # Per-c-tile local group selectors (8 local groups per 128-channel
    # tile): smat[t]: channel->group averaging; stmat[t]: group->channel.
    s_d = nc.declare_dram_parameter("smat", [CT * P, P], f32, isOutput=False)
    st_d = nc.declare_dram_parameter("stmat", [CT * P, P], f32, isOutput=False)
    out_d = nc.declare_dram_parameter("out", [C, W], f32, isOutput=True)
    if DEBUG:
        hdbg_d = nc.declare_dram_parameter("hdbg", [C, W], f32, isOutput=True)
        qdbg_d = nc.declare_dram_parameter("qdbg", [C, W], f32, isOutput=True)
        kdbg_d = nc.declare_dram_parameter("kdbg", [C, W], f32, isOutput=True)
        vtdbg_d = nc.declare_dram_parameter("vtdbg", [W, C], f32, isOutput=True)
        attdbg_d = nc.declare_dram_parameter("attdbg", [P, W], f32, isOutput=True)
        attTdbg_d = nc.declare_dram_parameter("attTdbg", [P, IT, 512], f32, isOutput=True)
        hodbg_d = nc.declare_dram_parameter("hodbg", [C, 512], f32, isOutput=True)

    with tile.TileContext(nc) as tc:
        with (
            tc.tile_pool(name="singles", bufs=1) as singles,
            tc.tile_pool(name="ps_small", bufs=8, space="PSUM") as ps_small,
            tc.tile_pool(name="qk", bufs=1) as qkp,
            tc.tile_pool(name="vt", bufs=1) as vtp,
            tc.tile_pool(name="gn", bufs=2) as gnp,
        ):
            # Pool nesting (LIFO): wqkv > hp > xp. x DMAs are emitted first
            # so they lead the sync queues; weight DMAs follow.
            wqkv_cm = tc.tile_pool(name="wqkv", bufs=1)
            wqkv = wqkv_cm.__enter__()
            wq_sb_l = [wqkv.tile([P, C], f32r, name=f"wqT{t}") for t in range(CT)]
            wk_sb_l = [wqkv.tile([P, C], f32r, name=f"wkT{t}") for t in range(CT)]
            wv_sb_l = [wqkv.tile([P, C], f32r, name=f"wvT{t}") for t in range(CT)]
            hp_cm = tc.tile_pool(name="hp", bufs=1)
            hp = hp_cm.__enter__()
            h_sb = [hp.tile([P, W], f32r, name=f"h{t}") for t in range(CT)]
            xp_cm = tc.tile_pool(name="xp", bufs=1)
            xp = xp_cm.__enter__()
            x_sb = [xp.tile([P, W], f32, name=f"x{t}") for t in range(CT)]

            # ---- singles (tiny DMAs first so they aren't queued behind x) ----
            ident = singles.tile([P, P], f32, name="ident")
            make_identity(nc, ident)
            ident_r = singles.tile([P, P], f32r, name="ident_r")
            nc.vector.tensor_copy(out=ident_r, in_=ident)
            eps_t = singles.tile([P, 1], f32, name="eps_t")
            nc.vector.memset(eps_t, EPS)
            s_sb = singles.tile([P, CT, P], f32, name="s_sb")
            nc.sync.dma_start(out=s_sb, in_=s_d.rearrange("(t p) g -> p t g", p=P))
            st_sb = singles.tile([P, CT, P], f32, name="st_sb")
            nc.sync.dma_start(out=st_sb, in_=st_d.rearrange("(t p) c -> p t c", p=P))
            bq_sb = singles.tile([P, CT], f32, name="bq_sb")
            nc.sync.dma_start(out=bq_sb, in_=bq_d.rearrange("(t p) o -> p (t o)", p=P))
            bk_sb = singles.tile([P, CT], f32, name="bk_sb")
            nc.sync.dma_start(out=bk_sb, in_=bk_d.rearrange("(t p) o -> p (t o)", p=P))
            bp_sb = singles.tile([P, CT], f32, name="bp_sb")
            nc.sync.dma_start(out=bp_sb, in_=bp_d.rearrange("(t p) o -> p (t o)", p=P))
            gam_sb = singles.tile([P, CT], f32, name="gam_sb")
            nc.sync.dma_start(out=gam_sb, in_=gam_d.rearrange("(t p) o -> p (t o)", p=P))
            bet_sb = singles.tile([P, CT], f32, name="bet_sb")
            nc.sync.dma_start(out=bet_sb, in_=bet_d.rearrange("(t p) o -> p (t o)", p=P))
            wpT_sb = [singles.tile([P, C], f32r, name=f"wpT{t}") for t in range(CT)]
            for t in range(CT):
                for hw in range(2):
                    nc.sync.dma_start(
                        out=x_sb[t][:, hw * 1024:(hw + 1) * 1024],
                        in_=x_d[t * P:(t + 1) * P, hw * 1024:(hw + 1) * 1024])
            for t in range(CT):
                nc.sync.dma_start(out=wq_sb_l[t],
                                  in_=wqT_d[t * P:(t + 1) * P, :].bitcast(f32r))
            for t in range(CT):
                nc.sync.dma_start(out=wk_sb_l[t],
                                  in_=wkT_d[t * P:(t + 1) * P, :].bitcast(f32r))
            for t in range(CT):
                nc.sync.dma_start(out=wv_sb_l[t],
                                  in_=wvT_d[t * P:(t + 1) * P, :].bitcast(f32r))

            if True:
                pass
                # ===== GroupNorm: stats pass for all tiles first (keeps
                # DVE free of head-of-line blocking on the per-tile chains)
                st2_l = []
                for t in range(CT):
                    stats = gnp.tile([P, NW, 6], f32, tag="bnstats", name=f"bns{t}")
                    for sg in range(NW):
                        nc.vector.bn_stats(out=stats[:, sg, :],
                                           in_=x_sb[t][:, sg * 512:(sg + 1) * 512])
                    mv = gnp.tile([P, 2], f32, tag="mv", name=f"mv{t}")
                    nc.vector.bn_aggr(out=mv, in_=stats)
                    st2 = gnp.tile([P, 2], f32, tag=f"st2_{t}", name=f"st2_{t}")
                    nc.vector.tensor_copy(out=st2[:, 0:1], in_=mv[:, 0:1])
                    nc.vector.tensor_tensor(out=st2[:, 1:2], in0=mv[:, 0:1],
                                            in1=mv[:, 0:1], op=mult)
                    nc.vector.tensor_add(out=st2[:, 1:2], in0=st2[:, 1:2],
                                         in1=mv[:, 1:2])
                    st2_l.append(st2)
                for t in range(CT):
                    st2 = st2_l[t]
                    ps_g = ps_small.tile([P, 2], f32, tag="ps512", name=f"ps_g{t}")
                    nc.tensor.matmul(ps_g[:], lhsT=s_sb[:, t, :], rhs=st2,
                                     start=True, stop=True)
                    gsr = gnp.tile([P, 2], f32, tag="gsr", name=f"gsr{t}")
                    nc.vector.tensor_copy(out=gsr[:8, :], in_=ps_g[:8, :])
                    gs2 = gnp.tile([P, 2], f32, tag="gs2", name=f"gs2_{t}")
                    nc.vector.memset(gs2, 0.0)
                    nc.vector.tensor_copy(out=gs2[:8, 0:1], in_=gsr[:8, 0:1])
                    nc.vector.tensor_tensor(out=gs2[:8, 1:2], in0=gsr[:8, 0:1],
                                            in1=gsr[:8, 0:1], op=mult)
                    nc.vector.tensor_tensor(out=gs2[:8, 1:2], in0=gsr[:8, 1:2],
                                            in1=gs2[:8, 1:2], op=subtract)
                    nc.scalar.activation(out=gs2[:8, 1:2], in_=gs2[:8, 1:2],
                                         func=Sqrt, bias=eps_t[:8], scale=1.0)
                    nc.vector.reciprocal(gs2[:8, 1:2], gs2[:8, 1:2])
                    ps_bc = ps_small.tile([P, 2], f32, tag="ps512", name=f"psbc{t}")
                    nc.tensor.matmul(ps_bc[:], lhsT=st_sb[:, t, :],
                                     rhs=gs2, start=True, stop=True)
                    bca = gnp.tile([P, 2], f32, tag="bca", name=f"bca{t}")
                    nc.vector.tensor_copy(out=bca, in_=ps_bc)
                    alph = gnp.tile([P, 1], f32, tag=f"alph{t}", name=f"alph{t}")
                    nc.vector.tensor_tensor(out=alph, in0=bca[:, 1:2],
                                            in1=gam_sb[:, t:t + 1], op=mult)
                    beta = gnp.tile([P, 1], f32, tag=f"beta{t}", name=f"beta{t}")
                    nc.vector.tensor_tensor(out=beta, in0=bca[:, 0:1],
                                            in1=alph, op=mult)
                    nc.vector.tensor_tensor(out=beta, in0=bet_sb[:, t:t + 1],
                                            in1=beta, op=subtract)
                    if t % 2 == 0:
                        nc.scalar.activation(out=h_sb[t], in_=x_sb[t],
                                             func=Ident, scale=alph, bias=beta)
                    else:
                        nc.vector.tensor_scalar(out=h_sb[t], in0=x_sb[t],
                                                scalar1=alph, scalar2=beta,
                                                op0=mult, op1=add)
                xp_cm.__exit__(None, None, None)

                # ================= QKV =================
                q_sb = [qkp.tile([P, W], f32r, name=f"q{t}") for t in range(CT)]
                k_sb = [qkp.tile([P, W], f32r, name=f"k{t}") for t in range(CT)]
                vT_sb = [vtp.tile([P, C], f32r, name=f"vT{j}") for j in range(IT)]

                if True:
                    wqT_sb, wkT_sb, wvT_sb = wq_sb_l, wk_sb_l, wv_sb_l

                    # chains: (kind, idx) kind in q/k/v; process 8 at a time,
                    # ct-phase-major so the PE has a full phase of work as
                    # each h[ct] lands.
                    chains = ([("q", ot, jc) for ot in range(CT) for jc in range(NW)]
                              + [("k", ot, jc) for ot in range(CT) for jc in range(NW)]
                              + [("v", jt, 0) for jt in range(IT)])
                    for g8 in range(0, len(chains), 8):
                        grp = chains[g8:g8 + 8]
                        pss = {}
                        for ch in grp:
                            pss[ch] = ps_small.tile(
                                [P, 512], f32, tag="ps512",
                                name=f"psqkv{ch[0]}{ch[1]}_{ch[2]}")
                        for ct in range(CT):
                            for ch in grp:
                                kind, a, b = ch
                                if kind == "q":
                                    lhsT = wqT_sb[ct][:, a * P:(a + 1) * P]
                                    rhs = h_sb[ct][:, b * 512:(b + 1) * 512]
                                elif kind == "k":
                                    lhsT = wkT_sb[ct][:, a * P:(a + 1) * P]
                                    rhs = h_sb[ct][:, b * 512:(b + 1) * 512]
                                else:
                                    lhsT = h_sb[ct][:, a * P:(a + 1) * P]
                                    rhs = wvT_sb[ct]
                                nc.tensor.matmul(pss[ch][:], lhsT=lhsT, rhs=rhs,
                                                 start=(ct == 0),
                                                 stop=(ct == CT - 1))
                        for ch in grp:
                            kind, a, b = ch
                            if kind == "q":
                                nc.scalar.activation(
                                    out=q_sb[a][:, b * 512:(b + 1) * 512],
                                    in_=pss[ch], func=Ident,
                                    bias=bq_sb[:, a:a + 1], scale=1.0)
                            elif kind == "k":
                                nc.scalar.activation(
                                    out=k_sb[a][:, b * 512:(b + 1) * 512],
                                    in_=pss[ch], func=Ident,
                                    bias=bk_sb[:, a:a + 1], scale=1.0)
                            else:
                                nc.vector.tensor_copy(out=vT_sb[a], in_=pss[ch])

                    if DEBUG:
                        for t in range(CT):
                            nc.sync.dma_start(out=hdbg_d[t * P:(t + 1) * P, :].bitcast(f32r), in_=h_sb[t])
                            nc.sync.dma_start(out=qdbg_d[t * P:(t + 1) * P, :].bitcast(f32r), in_=q_sb[t])
                            nc.sync.dma_start(out=kdbg_d[t * P:(t + 1) * P, :].bitcast(f32r), in_=k_sb[t])
                        for j in range(IT):
                            nc.sync.dma_start(out=vtdbg_d[j * P:(j + 1) * P, :].bitcast(f32r), in_=vT_sb[j])

            hp_cm.__exit__(None, None, None)
            wqkv_cm.__exit__(None, None, None)
            for t in range(CT):
                nc.sync.dma_start(out=wpT_sb[t],
                                  in_=wpT_d[t * P:(t + 1) * P, :].bitcast(f32r))

            # ================= Attention + proj =================
            attn_pools = (
                tc.tile_pool(name="attT", bufs=1),
                tc.tile_pool(name="att", bufs=2),
                tc.tile_pool(name="hop", bufs=2),
                tc.tile_pool(name="outp", bufs=2),
                tc.tile_pool(name="xs", bufs=2),
            )
            attTp = attn_pools[0].__enter__()
            attp = attn_pools[1].__enter__()
            hop = attn_pools[2].__enter__()
            outp = attn_pools[3].__enter__()
            xsp = attn_pools[4].__enter__()
            attT = attTp.tile([P, IT, 512], f32r, name="attT")
            att_tiles = {}

            def emit_scores(it):
                att = attp.tile([P, W], f32r, tag="att", name=f"att{it}")
                att_tiles[it] = att
                srows = gnp.tile([P, NW], f32, tag="srows", name=f"srows{it}")
                for jc in range(NW):
                    ps_s = ps_small.tile([P, 512], f32, tag="ps512",
                                         name=f"sc{it}_{jc}")
                    for ct in range(CT):
                        nc.tensor.matmul(
                            ps_s[:],
                            lhsT=q_sb[ct][:, it * P:(it + 1) * P],
                            rhs=k_sb[ct][:, jc * 512:(jc + 1) * 512],
                            start=(ct == 0), stop=(ct == CT - 1))
                    nc.scalar.activation(out=att[:, jc * 512:(jc + 1) * 512],
                                         in_=ps_s, func=Exp,
                                         bias=0.0, scale=1.0,
                                         accum_out=srows[:, jc:jc + 1])
                srow = gnp.tile([P, 1], f32, tag="srow", name=f"srow{it}")
                nc.vector.reduce_sum(srow, srows, axis=mybir.AxisListType.X)
                rec = gnp.tile([P, 1], f32, tag="rec", name=f"rec{it}")
                nc.vector.reciprocal(rec, srow)
                nc.vector.tensor_scalar_mul(att, att, rec)

            def emit_transposes(it):
                att = att_tiles.pop(it)
                s = it % IGRP
                for jt4 in range(4):
                    ps_t = ps_small.tile([P, 512], f32r, tag="ps512",
                                         name=f"pst{it}_{jt4}")
                    for j4 in range(4):
                        jt = jt4 * 4 + j4
                        nc.tensor.transpose(
                            ps_t[:, j4 * P:(j4 + 1) * P],
                            att[:, jt * P:(jt + 1) * P], ident_r)
                    nc.vector.tensor_copy(
                        out=attT[:, jt4 * 4:jt4 * 4 + 4, s * P:(s + 1) * P],
                        in_=ps_t.rearrange("p (a b) -> p a b", a=4))

            def emit_ho_proj(g):
                ho_sb = []
                for ct in range(CT):
                    ps_ho = ps_small.tile([P, 512], f32, tag="ps512",
                                          name=f"psho{g}_{ct}")
                    for jt in range(IT):
                        nc.tensor.matmul(
                            ps_ho[:],
                            lhsT=vT_sb[jt][:, ct * P:(ct + 1) * P],
                            rhs=attT[:, jt, :],
                            start=(jt == 0), stop=(jt == IT - 1))
                    ho = hop.tile([P, 512], f32r, tag=f"ho{ct}", name=f"ho{g}_{ct}")
                    nc.vector.tensor_copy(out=ho, in_=ps_ho)
                    ho_sb.append(ho)
                for ot in range(CT):
                    ps_o = ps_small.tile([P, 512], f32, tag="ps512",
                                         name=f"pso{g}_{ot}")
                    for ct in range(CT):
                        nc.tensor.matmul(
                            ps_o[:],
                            lhsT=wpT_sb[ct][:, ot * P:(ot + 1) * P],
                            rhs=ho_sb[ct],
                            start=(ct == 0), stop=(ct == CT - 1))
                    xs = xsp.tile([P, 512], f32, tag="xs", name=f"xs{g}_{ot}")
                    nc.sync.dma_start(
                        out=xs,
                        in_=x_d[ot * P:(ot + 1) * P, g * 512:(g + 1) * 512])
                    tmp = outp.tile([P, 512], f32, tag="tmp", name=f"tmp{g}_{ot}")
                    nc.vector.tensor_add(out=tmp, in0=ps_o, in1=xs)
                    osb = outp.tile([P, 512], f32, tag="osb", name=f"osb{g}_{ot}")
                    nc.scalar.activation(out=osb, in_=tmp, func=Ident,
                                         bias=bp_sb[:, ot:ot + 1], scale=1.0)
                    nc.sync.dma_start(
                        out=out_d[ot * P:(ot + 1) * P, g * 512:(g + 1) * 512],
                        in_=osb)

            for step in range(IT + 1):
                if step < IT:
                    emit_scores(step)
                if step >= 1:
                    emit_transposes(step - 1)
                if step % IGRP == 0 and step >= IGRP:
                    emit_ho_proj(step // IGRP - 1)
            for pcm in reversed(attn_pools):
                pcm.__exit__(None, None, None)

    nc.finalize()
    return nc


@functools.lru_cache(maxsize=1)
def _built():
    return _build_nc()


def kernel(x, gn_gamma, gn_beta, wq, bq, wk, bk, wv, bv, wp, bp):
    global LAST_EXEC_NS, LAST_TRACE_PATH
    from concourse.bass_utils import run_bass_kernel_spmd

    x = np.asarray(x, dtype=np.float32)
    scale = float(C) ** -0.5
    f = np.float32
    wqT = np.ascontiguousarray(np.asarray(wq, f).T * f(scale))
    wkT = np.ascontiguousarray(np.asarray(wk, f).T)
    wvT = np.ascontiguousarray(np.asarray(wv, f).T)
    wpT = np.ascontiguousarray(np.asarray(wp, f).T)
    bq_s = (np.asarray(bq, f) * f(scale)).reshape(C, 1)
    bk_c = np.asarray(bk, f).reshape(C, 1)
    # v bias folds through the (row-stochastic) attention average into proj
    bp_eff = (np.asarray(bp, f) + np.asarray(wp, f) @ np.asarray(bv, f)).reshape(C, 1)
    gam = np.asarray(gn_gamma, f).reshape(C, 1)
    bet = np.asarray(gn_beta, f).reshape(C, 1)

    gsz = C // G
    smat = np.zeros((CT * P, P), dtype=f)
    stmat = np.zeros((CT * P, P), dtype=f)
    for t in range(CT):
        for cl in range(P):
            smat[t * P + cl, cl // gsz] = 1.0 / gsz
            stmat[t * P + cl // gsz, cl] = 1.0

    shared = dict(wqT=wqT, wkT=wkT, wvT=wvT, wpT=wpT, bq=bq_s, bk=bk_c,
                  bp=bp_eff, gam=gam, bet=bet, smat=smat, stmat=stmat)
    in_maps = [dict(x=np.ascontiguousarray(x[i]), **shared) for i in range(B)]

    nc = _built()
    res = run_bass_kernel_spmd(nc, in_maps, list(range(B)), trace=TRACE)
    if TRACE:
        LAST_EXEC_NS = res.exec_time_ns
        if res.instructions_and_trace is not None:
            LAST_TRACE_PATH = res.instructions_and_trace[1]
    return np.stack([res.results[i]["out"] for i in range(B)], axis=0)


# revision 20
# speedup vs baseline: 1.0922x; 1.0922x over previous
"""AttnBlock (GroupNorm -> QKV -> full attention -> proj + residual) on 8
Trainium2 NeuronCores, data-parallel over the batch dimension (b=8, one
sample per core).

Layouts per core (sample):
  x:  (c=512, w=2048) fp32, channel tiles of 128 partitions.
  h:  GroupNorm(x) in f32r (feeds all matmuls).
  q,k: (c, w) f32r; vT: (w, c) f32r computed directly transposed.
  scores_i = q[:, i-tile].T @ k  -> PSUM (128, 2048); softmax without
  max-subtraction (scores are O(1) for this problem); exp fused with
  row-sum via ACT accum_out; normalization fused into the PE transpose by
  using diag(1/rowsum) as the transpose identity.
  ho = sum_j vT.T @ attT, proj with host-transposed wp, bias folded with
  wp @ bv, residual added from a streamed x slice.
"""

import functools

import numpy as np

B = 8
C = 512
W = 2048
G = 32
EPS = 1e-6
P = 128
CT = C // P          # 4 channel tiles
NW = W // 512        # 4 w-chunks of 512
IT = W // P          # 16 i-tiles
IGRP = 4             # i-tiles per ho/proj group
NG = IT // IGRP      # 4 groups

TRACE = False
DEBUG = False
LAST_EXEC_NS = None
LAST_TRACE_PATH = None


def _build_nc():
    import concourse.bass as bass
    import concourse.mybir as mybir
    import concourse.tile as tile
    from concourse import bacc
    from concourse.masks import make_identity

    f32 = mybir.dt.float32
    f32r = mybir.dt.float32r
    Ident = mybir.ActivationFunctionType.Identity
    Exp = mybir.ActivationFunctionType.Exp
    Sqrt = mybir.ActivationFunctionType.Sqrt
    mult = mybir.AluOpType.mult
    add = mybir.AluOpType.add
    subtract = mybir.AluOpType.subtract

    nc = bacc.Bacc()

    x_d = nc.declare_dram_parameter("x", [C, W], f32, isOutput=False)
    wqT_d = nc.declare_dram_parameter("wqT", [C, C], f32, isOutput=False)
    wkT_d = nc.declare_dram_parameter("wkT", [C, C], f32, isOutput=False)
    wvT_d = nc.declare_dram_parameter("wvT", [C, C], f32, isOutput=False)
    wpT_d = nc.declare_dram_parameter("wpT", [C, C], f32, isOutput=False)
    # One packed small-constant parameter (partition-major):
    # [0:512] per-tile group-avg selector S, [512:1024] selector-back ST,
    # then bq, bk, bp, gam, bet (CT cols each).
    aux_d = nc.declare_dram_parameter("aux", [P, 1044], f32, isOutput=False)
    out_d = nc.declare_dram_parameter("out", [C, W], f32, isOutput=True)
    if DEBUG:
        hdbg_d = nc.declare_dram_parameter("hdbg", [C, W], f32, isOutput=True)
        qdbg_d = nc.declare_dram_parameter("qdbg", [C, W], f32, isOutput=True)
        kdbg_d = nc.declare_dram_parameter("kdbg", [C, W], f32, isOutput=True)
        vtdbg_d = nc.declare_dram_parameter("vtdbg", [W, C], f32, isOutput=True)
        attdbg_d = nc.declare_dram_parameter("attdbg", [P, W], f32, isOutput=True)
        attTdbg_d = nc.declare_dram_parameter("attTdbg", [P, IT, 512], f32, isOutput=True)
        hodbg_d = nc.declare_dram_parameter("hodbg", [C, 512], f32, isOutput=True)

    with tile.TileContext(nc) as tc:
        with (
            tc.tile_pool(name="singles", bufs=1) as singles,
            tc.tile_pool(name="ps_small", bufs=8, space="PSUM") as ps_small,
            tc.tile_pool(name="qk", bufs=1) as qkp,
            tc.tile_pool(name="vt", bufs=1) as vtp,
            tc.tile_pool(name="gn", bufs=2) as gnp,
        ):
            # Pool nesting (LIFO): wqkv > hp > xp. x DMAs are emitted first
            # so they lead the sync queues; weight DMAs follow.
            wqkv_cm = tc.tile_pool(name="wqkv", bufs=1)
            wqkv = wqkv_cm.__enter__()
            wq_sb = wqkv.tile([P, CT, C], f32r, name="wq_sb")
            wk_sb = wqkv.tile([P, CT, C], f32r, name="wk_sb")
            wv_sb = wqkv.tile([P, CT, C], f32r, name="wv_sb")
            wq_sb_l = [wq_sb[:, t, :] for t in range(CT)]
            wk_sb_l = [wk_sb[:, t, :] for t in range(CT)]
            wv_sb_l = [wv_sb[:, t, :] for t in range(CT)]
            hp_cm = tc.tile_pool(name="hp", bufs=1)
            hp = hp_cm.__enter__()
            h_sb = [hp.tile([P, W], f32r, name=f"h{t}") for t in range(CT)]
            xp_cm = tc.tile_pool(name="xp", bufs=1)
            xp = xp_cm.__enter__()
            x_sb = [xp.tile([P, W], f32, name=f"x{t}") for t in range(CT)]

            # ---- singles (tiny DMAs first so they aren't queued behind x) ----
            ident = singles.tile([P, P], f32, name="ident")
            make_identity(nc, ident)
            ident_r = singles.tile([P, P], f32r, name="ident_r")
            nc.vector.tensor_copy(out=ident_r, in_=ident)
            eps_t = singles.tile([P, 1], f32, name="eps_t")
            nc.vector.memset(eps_t, EPS)
            aux_sb = singles.tile([P, 1044], f32, name="aux_sb")
            nc.sync.dma_start(out=aux_sb, in_=aux_d[:, :])
            s_sb = aux_sb[:, 0:512].rearrange("p (t g) -> p t g", t=CT)
            st_sb = aux_sb[:, 512:1024].rearrange("p (t c) -> p t c", t=CT)
            bq_sb = aux_sb[:, 1024:1028]
            bk_sb = aux_sb[:, 1028:1032]
            bp_sb = aux_sb[:, 1032:1036]
            gam_sb = aux_sb[:, 1036:1040]
            bet_sb = aux_sb[:, 1040:1044]
            wpT_all = singles.tile([P, CT, C], f32r, name="wpT_all")
            wpT_sb = [wpT_all[:, t, :] for t in range(CT)]
            for t in range(CT):
                nc.sync.dma_start(out=x_sb[t], in_=x_d[t * P:(t + 1) * P, :])
            nc.sync.dma_start(
                out=wq_sb, in_=wqT_d.rearrange("(t p) c -> p t c", p=P).bitcast(f32r))
            nc.sync.dma_start(
                out=wk_sb, in_=wkT_d.rearrange("(t p) c -> p t c", p=P).bitcast(f32r))
            nc.sync.dma_start(
                out=wv_sb, in_=wvT_d.rearrange("(t p) c -> p t c", p=P).bitcast(f32r))

            if True:
                pass
                # ===== GroupNorm: stats pass for all tiles first (keeps
                # DVE free of head-of-line blocking on the per-tile chains)
                st2_l = []
                for t in range(CT):
                    stats = gnp.tile([P, NW, 6], f32, tag="bnstats", name=f"bns{t}")
                    for sg in range(NW):
                        nc.vector.bn_stats(out=stats[:, sg, :],
                                           in_=x_sb[t][:, sg * 512:(sg + 1) * 512])
                    mv = gnp.tile([P, 2], f32, tag="mv", name=f"mv{t}")
                    nc.vector.bn_aggr(out=mv, in_=stats)
                    st2 = gnp.tile([P, 2], f32, tag=f"st2_{t}", name=f"st2_{t}")
                    nc.vector.tensor_copy(out=st2[:, 0:1], in_=mv[:, 0:1])
                    nc.vector.tensor_tensor(out=st2[:, 1:2], in0=mv[:, 0:1],
                                            in1=mv[:, 0:1], op=mult)
                    nc.vector.tensor_add(out=st2[:, 1:2], in0=st2[:, 1:2],
                                         in1=mv[:, 1:2])
                    st2_l.append(st2)
                for t in range(CT):
                    st2 = st2_l[t]
                    ps_g = ps_small.tile([P, 2], f32, tag="ps512", name=f"ps_g{t}")
                    nc.tensor.matmul(ps_g[:], lhsT=s_sb[:, t, :], rhs=st2,
                                     start=True, stop=True)
                    gsr = gnp.tile([P, 2], f32, tag="gsr", name=f"gsr{t}")
                    nc.vector.tensor_copy(out=gsr[:8, :], in_=ps_g[:8, :])
                    gs2 = gnp.tile([P, 2], f32, tag="gs2", name=f"gs2_{t}")
                    nc.vector.memset(gs2, 0.0)
                    nc.vector.tensor_copy(out=gs2[:8, 0:1], in_=gsr[:8, 0:1])
                    nc.vector.tensor_tensor(out=gs2[:8, 1:2], in0=gsr[:8, 0:1],
                                            in1=gsr[:8, 0:1], op=mult)
                    nc.vector.tensor_tensor(out=gs2[:8, 1:2], in0=gsr[:8, 1:2],
                                            in1=gs2[:8, 1:2], op=subtract)
                    nc.scalar.activation(out=gs2[:8, 1:2], in_=gs2[:8, 1:2],
                                         func=Sqrt, bias=eps_t[:8], scale=1.0)
                    nc.vector.reciprocal(gs2[:8, 1:2], gs2[:8, 1:2])
                    ps_bc = ps_small.tile([P, 2], f32, tag="ps512", name=f"psbc{t}")
                    nc.tensor.matmul(ps_bc[:], lhsT=st_sb[:, t, :],
                                     rhs=gs2, start=True, stop=True)
                    bca = gnp.tile([P, 2], f32, tag="bca", name=f"bca{t}")
                    nc.vector.tensor_copy(out=bca, in_=ps_bc)
                    alph = gnp.tile([P, 1], f32, tag=f"alph{t}", name=f"alph{t}")
                    nc.vector.tensor_tensor(out=alph, in0=bca[:, 1:2],
                                            in1=gam_sb[:, t:t + 1], op=mult)
                    beta = gnp.tile([P, 1], f32, tag=f"beta{t}", name=f"beta{t}")
                    nc.vector.tensor_tensor(out=beta, in0=bca[:, 0:1],
                                            in1=alph, op=mult)
                    nc.vector.tensor_tensor(out=beta, in0=bet_sb[:, t:t + 1],
                                            in1=beta, op=subtract)
                    if t % 2 == 0:
                        nc.scalar.activation(out=h_sb[t], in_=x_sb[t],
                                             func=Ident, scale=alph, bias=beta)
                    else:
                        nc.vector.tensor_scalar(out=h_sb[t], in0=x_sb[t],
                                                scalar1=alph, scalar2=beta,
                                                op0=mult, op1=add)
                xp_cm.__exit__(None, None, None)

                # ================= QKV =================
                q_sb = [qkp.tile([P, W], f32r, name=f"q{t}") for t in range(CT)]
                k_sb = [qkp.tile([P, W], f32r, name=f"k{t}") for t in range(CT)]
                vT_sb = [vtp.tile([P, C], f32r, name=f"vT{j}") for j in range(IT)]

                if True:
                    wqT_sb, wkT_sb, wvT_sb = wq_sb_l, wk_sb_l, wv_sb_l

                    # chains: (kind, idx) kind in q/k/v; process 8 at a time,
                    # ct-phase-major so the PE has a full phase of work as
                    # each h[ct] lands.
                    chains = ([("q", ot, jc) for ot in range(CT) for jc in range(NW)]
                              + [("k", ot, jc) for ot in range(CT) for jc in range(NW)]
                              + [("v", jt, 0) for jt in range(IT)])
                    for g8 in range(0, len(chains), 8):
                        grp = chains[g8:g8 + 8]
                        pss = {}
                        for ch in grp:
                            pss[ch] = ps_small.tile(
                                [P, 512], f32, tag="ps512",
                                name=f"psqkv{ch[0]}{ch[1]}_{ch[2]}")
                        for ct in range(CT):
                            for ch in grp:
                                kind, a, b = ch
                                if kind == "q":
                                    lhsT = wqT_sb[ct][:, a * P:(a + 1) * P]
                                    rhs = h_sb[ct][:, b * 512:(b + 1) * 512]
                                elif kind == "k":
                                    lhsT = wkT_sb[ct][:, a * P:(a + 1) * P]
                                    rhs = h_sb[ct][:, b * 512:(b + 1) * 512]
                                else:
                                    lhsT = h_sb[ct][:, a * P:(a + 1) * P]
                                    rhs = wvT_sb[ct]
                                nc.tensor.matmul(pss[ch][:], lhsT=lhsT, rhs=rhs,
                                                 start=(ct == 0),
                                                 stop=(ct == CT - 1))
                        for ch in grp:
                            kind, a, b = ch
                            if kind == "q":
                                nc.scalar.activation(
                                    out=q_sb[a][:, b * 512:(b + 1) * 512],
                                    in_=pss[ch], func=Ident,
                                    bias=bq_sb[:, a:a + 1], scale=1.0)
                            elif kind == "k":
                                nc.scalar.activation(
                                    out=k_sb[a][:, b * 512:(b + 1) * 512],
                                    in_=pss[ch], func=Ident,
                                    bias=bk_sb[:, a:a + 1], scale=1.0)
                            else:
                                nc.vector.tensor_copy(out=vT_sb[a], in_=pss[ch])

                    if DEBUG:
                        for t in range(CT):
                            nc.sync.dma_start(out=hdbg_d[t * P:(t + 1) * P, :].bitcast(f32r), in_=h_sb[t])
                            nc.sync.dma_start(out=qdbg_d[t * P:(t + 1) * P, :].bitcast(f32r), in_=q_sb[t])
                            nc.sync.dma_start(out=kdbg_d[t * P:(t + 1) * P, :].bitcast(f32r), in_=k_sb[t])
                        for j in range(IT):
                            nc.sync.dma_start(out=vtdbg_d[j * P:(j + 1) * P, :].bitcast(f32r), in_=vT_sb[j])

            hp_cm.__exit__(None, None, None)
            wqkv_cm.__exit__(None, None, None)
            nc.sync.dma_start(
                out=wpT_all, in_=wpT_d.rearrange("(t p) c -> p t c", p=P).bitcast(f32r))

            # ================= Attention + proj =================
            attn_pools = (
                tc.tile_pool(name="attT", bufs=1),
                tc.tile_pool(name="att", bufs=2),
                tc.tile_pool(name="hop", bufs=2),
                tc.tile_pool(name="outp", bufs=2),
                tc.tile_pool(name="xs", bufs=2),
            )
            attTp = attn_pools[0].__enter__()
            attp = attn_pools[1].__enter__()
            hop = attn_pools[2].__enter__()
            outp = attn_pools[3].__enter__()
            xsp = attn_pools[4].__enter__()
            attT = attTp.tile([P, IT, 512], f32r, name="attT")
            att_tiles = {}

            def emit_scores(it):
                att = attp.tile([P, W], f32r, tag="att", name=f"att{it}")
                att_tiles[it] = att
                srows = gnp.tile([P, NW], f32, tag="srows", name=f"srows{it}")
                for jc in range(NW):
                    ps_s = ps_small.tile([P, 512], f32, tag="ps512",
                                         name=f"sc{it}_{jc}")
                    for ct in range(CT):
                        nc.tensor.matmul(
                            ps_s[:],
                            lhsT=q_sb[ct][:, it * P:(it + 1) * P],
                            rhs=k_sb[ct][:, jc * 512:(jc + 1) * 512],
                            start=(ct == 0), stop=(ct == CT - 1))
                    nc.scalar.activation(out=att[:, jc * 512:(jc + 1) * 512],
                                         in_=ps_s, func=Exp,
                                         bias=0.0, scale=1.0,
                                         accum_out=srows[:, jc:jc + 1])
                srow = gnp.tile([P, 1], f32, tag="srow", name=f"srow{it}")
                nc.vector.reduce_sum(srow, srows, axis=mybir.AxisListType.X)
                rec = gnp.tile([P, 1], f32, tag="rec", name=f"rec{it}")
                nc.vector.reciprocal(rec, srow)
                nc.vector.tensor_scalar_mul(att, att, rec)

            def emit_transposes(it):
                att = att_tiles.pop(it)
                s = it % IGRP
                for jt4 in range(4):
                    ps_t = ps_small.tile([P, 512], f32r, tag="ps512",
                                         name=f"pst{it}_{jt4}")
                    for j4 in range(4):
                        jt = jt4 * 4 + j4
                        nc.tensor.transpose(
                            ps_t[:, j4 * P:(j4 + 1) * P],
                            att[:, jt * P:(jt + 1) * P], ident_r)
                    nc.vector.tensor_copy(
                        out=attT[:, jt4 * 4:jt4 * 4 + 4, s * P:(s + 1) * P],
                        in_=ps_t.rearrange("p (a b) -> p a b", a=4))

            def emit_ho_proj(g):
                ho_sb = []
                for ct in range(CT):
                    ps_ho = ps_small.tile([P, 512], f32, tag="ps512",
                                          name=f"psho{g}_{ct}")
                    for jt in range(IT):
                        nc.tensor.matmul(
                            ps_ho[:],
                            lhsT=vT_sb[jt][:, ct * P:(ct + 1) * P],
                            rhs=attT[:, jt, :],
                            start=(jt == 0), stop=(jt == IT - 1))
                    ho = hop.tile([P, 512], f32r, tag=f"ho{ct}", name=f"ho{g}_{ct}")
                    nc.vector.tensor_copy(out=ho, in_=ps_ho)
                    ho_sb.append(ho)
                for ot in range(CT):
                    ps_o = ps_small.tile([P, 512], f32, tag="ps512",
                                         name=f"pso{g}_{ot}")
                    for ct in range(CT):
                        nc.tensor.matmul(
                            ps_o[:],
                            lhsT=wpT_sb[ct][:, ot * P:(ot + 1) * P],
                            rhs=ho_sb[ct],
                            start=(ct == 0), stop=(ct == CT - 1))
                    xs = xsp.tile([P, 512], f32, tag="xs", name=f"xs{g}_{ot}")
                    nc.sync.dma_start(
                        out=xs,
                        in_=x_d[ot * P:(ot + 1) * P, g * 512:(g + 1) * 512])
                    tmp = outp.tile([P, 512], f32, tag="tmp", name=f"tmp{g}_{ot}")
                    nc.vector.tensor_add(out=tmp, in0=ps_o, in1=xs)
                    osb = outp.tile([P, 512], f32, tag="osb", name=f"osb{g}_{ot}")
                    nc.scalar.activation(out=osb, in_=tmp, func=Ident,
                                         bias=bp_sb[:, ot:ot + 1], scale=1.0)
                    nc.sync.dma_start(
                        out=out_d[ot * P:(ot + 1) * P, g * 512:(g + 1) * 512],
                        in_=osb)

            for step in range(IT + 1):
                if step < IT:
                    emit_scores(step)
                if step >= 1:
                    emit_transposes(step - 1)
                if step % IGRP == 0 and step >= IGRP:
                    emit_ho_proj(step // IGRP - 1)
            for pcm in reversed(attn_pools):
                pcm.__exit__(None, None, None)

    nc.finalize()
    return nc


@functools.lru_cache(maxsize=1)
def _built():
    return _build_nc()


def kernel(x, gn_gamma, gn_beta, wq, bq, wk, bk, wv, bv, wp, bp):
    global LAST_EXEC_NS, LAST_TRACE_PATH
    from concourse.bass_utils import run_bass_kernel_spmd

    x = np.asarray(x, dtype=np.float32)
    scale = float(C) ** -0.5
    f = np.float32
    wqT = np.ascontiguousarray(np.asarray(wq, f).T * f(scale))
    wkT = np.ascontiguousarray(np.asarray(wk, f).T)
    wvT = np.ascontiguousarray(np.asarray(wv, f).T)
    wpT = np.ascontiguousarray(np.asarray(wp, f).T)
    bq_s = (np.asarray(bq, f) * f(scale)).reshape(C, 1)
    bk_c = np.asarray(bk, f).reshape(C, 1)
    # v bias folds through the (row-stochastic) attention average into proj
    bp_eff = (np.asarray(bp, f) + np.asarray(wp, f) @ np.asarray(bv, f)).reshape(C, 1)
    gam = np.asarray(gn_gamma, f).reshape(C, 1)
    bet = np.asarray(gn_beta, f).reshape(C, 1)

    gsz = C // G
    aux = np.zeros((P, 1044), dtype=f)
    for t in range(CT):
        for p in range(P):
            aux[p, t * P + p // gsz] = 1.0 / gsz          # S selector
            for cl in range(P):
                if p == cl // gsz:
                    aux[p, 512 + t * P + cl] = 1.0        # ST selector
    bqr = bq_s.reshape(CT, P).T
    bkr = bk_c.reshape(CT, P).T
    bpr = bp_eff.reshape(CT, P).T
    aux[:, 1024:1028] = bqr
    aux[:, 1028:1032] = bkr
    aux[:, 1032:1036] = bpr
    aux[:, 1036:1040] = gam.reshape(CT, P).T
    aux[:, 1040:1044] = bet.reshape(CT, P).T

    shared = dict(wqT=wqT, wkT=wkT, wvT=wvT, wpT=wpT, aux=aux)
    in_maps = [dict(x=np.ascontiguousarray(x[i]), **shared) for i in range(B)]

    nc = _built()
    res = run_bass_kernel_spmd(nc, in_maps, list(range(B)), trace=TRACE)
    if TRACE:
        LAST_EXEC_NS = res.exec_time_ns
        if res.instructions_and_trace is not None:
            LAST_TRACE_PATH = res.instructions_and_trace[1]
    return np.stack([res.results[i]["out"] for i in range(B)], axis=0)


# revision 21
# speedup vs baseline: 1.1062x; 1.0128x over previous
"""AttnBlock (GroupNorm -> QKV -> full attention -> proj + residual) on 8
Trainium2 NeuronCores, data-parallel over the batch dimension (b=8, one
sample per core).

Layouts per core (sample):
  x:  (c=512, w=2048) fp32, channel tiles of 128 partitions.
  h:  GroupNorm(x) in f32r (feeds all matmuls).
  q,k: (c, w) f32r; vT: (w, c) f32r computed directly transposed.
  scores_i = q[:, i-tile].T @ k  -> PSUM (128, 2048); softmax without
  max-subtraction (scores are O(1) for this problem); exp fused with
  row-sum via ACT accum_out; normalization fused into the PE transpose by
  using diag(1/rowsum) as the transpose identity.
  ho = sum_j vT.T @ attT, proj with host-transposed wp, bias folded with
  wp @ bv, residual added from a streamed x slice.
"""

import functools

import numpy as np

B = 8
C = 512
W = 2048
G = 32
EPS = 1e-6
P = 128
CT = C // P          # 4 channel tiles
NW = W // 512        # 4 w-chunks of 512
IT = W // P          # 16 i-tiles
IGRP = 4             # i-tiles per ho/proj group
NG = IT // IGRP      # 4 groups

TRACE = False
DEBUG = False
LAST_EXEC_NS = None
LAST_TRACE_PATH = None


def _build_nc():
    import concourse.bass as bass
    import concourse.mybir as mybir
    import concourse.tile as tile
    from concourse import bacc
    from concourse.masks import make_identity

    f32 = mybir.dt.float32
    f32r = mybir.dt.float32r
    Ident = mybir.ActivationFunctionType.Identity
    Exp = mybir.ActivationFunctionType.Exp
    Sqrt = mybir.ActivationFunctionType.Sqrt
    mult = mybir.AluOpType.mult
    add = mybir.AluOpType.add
    subtract = mybir.AluOpType.subtract

    nc = bacc.Bacc()

    x_d = nc.declare_dram_parameter("x", [C, W], f32, isOutput=False)
    # weights pre-arranged host-side to partition-major [P, CT*C]
    wqT_d = nc.declare_dram_parameter("wqT", [P, CT * C], f32, isOutput=False)
    wkT_d = nc.declare_dram_parameter("wkT", [P, CT * C], f32, isOutput=False)
    wvT_d = nc.declare_dram_parameter("wvT", [P, CT * C], f32, isOutput=False)
    wpT_d = nc.declare_dram_parameter("wpT", [P, CT * C], f32, isOutput=False)
    # One packed small-constant parameter (partition-major):
    # [0:512] per-tile group-avg selector S, [512:1024] selector-back ST,
    # then bq, bk, bp, gam, bet (CT cols each).
    aux_d = nc.declare_dram_parameter("aux", [P, 1044], f32, isOutput=False)
    out_d = nc.declare_dram_parameter("out", [C, W], f32, isOutput=True)
    if DEBUG:
        hdbg_d = nc.declare_dram_parameter("hdbg", [C, W], f32, isOutput=True)
        qdbg_d = nc.declare_dram_parameter("qdbg", [C, W], f32, isOutput=True)
        kdbg_d = nc.declare_dram_parameter("kdbg", [C, W], f32, isOutput=True)
        vtdbg_d = nc.declare_dram_parameter("vtdbg", [W, C], f32, isOutput=True)
        attdbg_d = nc.declare_dram_parameter("attdbg", [P, W], f32, isOutput=True)
        attTdbg_d = nc.declare_dram_parameter("attTdbg", [P, IT, 512], f32, isOutput=True)
        hodbg_d = nc.declare_dram_parameter("hodbg", [C, 512], f32, isOutput=True)

    with tile.TileContext(nc) as tc:
        with (
            tc.tile_pool(name="singles", bufs=1) as singles,
            tc.tile_pool(name="ps_small", bufs=8, space="PSUM") as ps_small,
            tc.tile_pool(name="qk", bufs=1) as qkp,
            tc.tile_pool(name="vt", bufs=1) as vtp,
            tc.tile_pool(name="gn", bufs=2) as gnp,
        ):
            # Pool nesting (LIFO): wqkv > hp > xp. x DMAs are emitted first
            # so they lead the sync queues; weight DMAs follow.
            wqkv_cm = tc.tile_pool(name="wqkv", bufs=1)
            wqkv = wqkv_cm.__enter__()
            wq_sb = wqkv.tile([P, CT, C], f32r, name="wq_sb")
            wk_sb = wqkv.tile([P, CT, C], f32r, name="wk_sb")
            wv_sb = wqkv.tile([P, CT, C], f32r, name="wv_sb")
            wq_sb_l = [wq_sb[:, t, :] for t in range(CT)]
            wk_sb_l = [wk_sb[:, t, :] for t in range(CT)]
            wv_sb_l = [wv_sb[:, t, :] for t in range(CT)]
            hp_cm = tc.tile_pool(name="hp", bufs=1)
            hp = hp_cm.__enter__()
            h_sb = [hp.tile([P, W], f32r, name=f"h{t}") for t in range(CT)]
            xp_cm = tc.tile_pool(name="xp", bufs=1)
            xp = xp_cm.__enter__()
            x_sb = [xp.tile([P, W], f32, name=f"x{t}") for t in range(CT)]

            # ---- singles (tiny DMAs first so they aren't queued behind x) ----
            ident = singles.tile([P, P], f32, name="ident")
            make_identity(nc, ident)
            ident_r = singles.tile([P, P], f32r, name="ident_r")
            nc.vector.tensor_copy(out=ident_r, in_=ident)
            eps_t = singles.tile([P, 1], f32, name="eps_t")
            nc.vector.memset(eps_t, EPS)
            aux_sb = singles.tile([P, 1044], f32, name="aux_sb")
            nc.sync.dma_start(out=aux_sb, in_=aux_d[:, :])
            s_sb = aux_sb[:, 0:512].rearrange("p (t g) -> p t g", t=CT)
            st_sb = aux_sb[:, 512:1024].rearrange("p (t c) -> p t c", t=CT)
            bq_sb = aux_sb[:, 1024:1028]
            bk_sb = aux_sb[:, 1028:1032]
            bp_sb = aux_sb[:, 1032:1036]
            gam_sb = aux_sb[:, 1036:1040]
            bet_sb = aux_sb[:, 1040:1044]
            wpT_all = singles.tile([P, CT, C], f32r, name="wpT_all")
            wpT_sb = [wpT_all[:, t, :] for t in range(CT)]
            for t in range(CT):
                nc.sync.dma_start(out=x_sb[t], in_=x_d[t * P:(t + 1) * P, :])
            nc.sync.dma_start(out=wq_sb, in_=wqT_d[:, :].bitcast(f32r))
            nc.sync.dma_start(out=wk_sb, in_=wkT_d[:, :].bitcast(f32r))
            nc.sync.dma_start(out=wv_sb, in_=wvT_d[:, :].bitcast(f32r))

            if True:
                pass
                # ===== GroupNorm: stats pass for all tiles first (keeps
                # DVE free of head-of-line blocking on the per-tile chains)
                st2_l = []
                for t in range(CT):
                    stats = gnp.tile([P, NW, 6], f32, tag="bnstats", name=f"bns{t}")
                    for sg in range(NW):
                        nc.vector.bn_stats(out=stats[:, sg, :],
                                           in_=x_sb[t][:, sg * 512:(sg + 1) * 512])
                    mv = gnp.tile([P, 2], f32, tag="mv", name=f"mv{t}")
                    nc.vector.bn_aggr(out=mv, in_=stats)
                    st2 = gnp.tile([P, 2], f32, tag=f"st2_{t}", name=f"st2_{t}")
                    nc.vector.tensor_copy(out=st2[:, 0:1], in_=mv[:, 0:1])
                    nc.vector.tensor_tensor(out=st2[:, 1:2], in0=mv[:, 0:1],
                                            in1=mv[:, 0:1], op=mult)
                    nc.vector.tensor_add(out=st2[:, 1:2], in0=st2[:, 1:2],
                                         in1=mv[:, 1:2])
                    st2_l.append(st2)
                for t in range(CT):
                    st2 = st2_l[t]
                    ps_g = ps_small.tile([P, 2], f32, tag="ps512", name=f"ps_g{t}")
                    nc.tensor.matmul(ps_g[:], lhsT=s_sb[:, t, :], rhs=st2,
                                     start=True, stop=True)
                    gsr = gnp.tile([P, 2], f32, tag="gsr", name=f"gsr{t}")
                    nc.vector.tensor_copy(out=gsr[:8, :], in_=ps_g[:8, :])
                    gs2 = gnp.tile([P, 2], f32, tag="gs2", name=f"gs2_{t}")
                    nc.vector.memset(gs2, 0.0)
                    nc.vector.tensor_copy(out=gs2[:8, 0:1], in_=gsr[:8, 0:1])
                    nc.vector.tensor_tensor(out=gs2[:8, 1:2], in0=gsr[:8, 0:1],
                                            in1=gsr[:8, 0:1], op=mult)
                    nc.vector.tensor_tensor(out=gs2[:8, 1:2], in0=gsr[:8, 1:2],
                                            in1=gs2[:8, 1:2], op=subtract)
                    nc.scalar.activation(out=gs2[:8, 1:2], in_=gs2[:8, 1:2],
                                         func=Sqrt, bias=eps_t[:8], scale=1.0)
                    nc.vector.reciprocal(gs2[:8, 1:2], gs2[:8, 1:2])
                    ps_bc = ps_small.tile([P, 2], f32, tag="ps512", name=f"psbc{t}")
                    nc.tensor.matmul(ps_bc[:], lhsT=st_sb[:, t, :],
                                     rhs=gs2, start=True, stop=True)
                    bca = gnp.tile([P, 2], f32, tag="bca", name=f"bca{t}")
                    nc.vector.tensor_copy(out=bca, in_=ps_bc)
                    alph = gnp.tile([P, 1], f32, tag=f"alph{t}", name=f"alph{t}")
                    nc.vector.tensor_tensor(out=alph, in0=bca[:, 1:2],
                                            in1=gam_sb[:, t:t + 1], op=mult)
                    beta = gnp.tile([P, 1], f32, tag=f"beta{t}", name=f"beta{t}")
                    nc.vector.tensor_tensor(out=beta, in0=bca[:, 0:1],
                                            in1=alph, op=mult)
                    nc.vector.tensor_tensor(out=beta, in0=bet_sb[:, t:t + 1],
                                            in1=beta, op=subtract)
                    if t % 2 == 0:
                        nc.scalar.activation(out=h_sb[t], in_=x_sb[t],
                                             func=Ident, scale=alph, bias=beta)
                    else:
                        nc.vector.tensor_scalar(out=h_sb[t], in0=x_sb[t],
                                                scalar1=alph, scalar2=beta,
                                                op0=mult, op1=add)
                xp_cm.__exit__(None, None, None)

                # ================= QKV =================
                q_sb = [qkp.tile([P, W], f32r, name=f"q{t}") for t in range(CT)]
                k_sb = [qkp.tile([P, W], f32r, name=f"k{t}") for t in range(CT)]
                vT_sb = [vtp.tile([P, C], f32r, name=f"vT{j}") for j in range(IT)]

                if True:
                    wqT_sb, wkT_sb, wvT_sb = wq_sb_l, wk_sb_l, wv_sb_l

                    # chains: (kind, idx) kind in q/k/v; process 8 at a time,
                    # ct-phase-major so the PE has a full phase of work as
                    # each h[ct] lands.
                    chains = ([("q", ot, jc) for ot in range(CT) for jc in range(NW)]
                              + [("k", ot, jc) for ot in range(CT) for jc in range(NW)]
                              + [("v", jt, 0) for jt in range(IT)])
                    for g8 in range(0, len(chains), 8):
                        grp = chains[g8:g8 + 8]
                        pss = {}
                        for ch in grp:
                            pss[ch] = ps_small.tile(
                                [P, 512], f32, tag="ps512",
                                name=f"psqkv{ch[0]}{ch[1]}_{ch[2]}")
                        for ct in range(CT):
                            for ch in grp:
                                kind, a, b = ch
                                if kind == "q":
                                    lhsT = wqT_sb[ct][:, a * P:(a + 1) * P]
                                    rhs = h_sb[ct][:, b * 512:(b + 1) * 512]
                                elif kind == "k":
                                    lhsT = wkT_sb[ct][:, a * P:(a + 1) * P]
                                    rhs = h_sb[ct][:, b * 512:(b + 1) * 512]
                                else:
                                    lhsT = h_sb[ct][:, a * P:(a + 1) * P]
                                    rhs = wvT_sb[ct]
                                nc.tensor.matmul(pss[ch][:], lhsT=lhsT, rhs=rhs,
                                                 start=(ct == 0),
                                                 stop=(ct == CT - 1))
                        for ch in grp:
                            kind, a, b = ch
                            if kind == "q":
                                nc.scalar.activation(
                                    out=q_sb[a][:, b * 512:(b + 1) * 512],
                                    in_=pss[ch], func=Ident,
                                    bias=bq_sb[:, a:a + 1], scale=1.0)
                            elif kind == "k":
                                nc.scalar.activation(
                                    out=k_sb[a][:, b * 512:(b + 1) * 512],
                                    in_=pss[ch], func=Ident,
                                    bias=bk_sb[:, a:a + 1], scale=1.0)
                            else:
                                nc.vector.tensor_copy(out=vT_sb[a], in_=pss[ch])

                    if DEBUG:
                        for t in range(CT):
                            nc.sync.dma_start(out=hdbg_d[t * P:(t + 1) * P, :].bitcast(f32r), in_=h_sb[t])
                            nc.sync.dma_start(out=qdbg_d[t * P:(t + 1) * P, :].bitcast(f32r), in_=q_sb[t])
                            nc.sync.dma_start(out=kdbg_d[t * P:(t + 1) * P, :].bitcast(f32r), in_=k_sb[t])
                        for j in range(IT):
                            nc.sync.dma_start(out=vtdbg_d[j * P:(j + 1) * P, :].bitcast(f32r), in_=vT_sb[j])

            hp_cm.__exit__(None, None, None)
            wqkv_cm.__exit__(None, None, None)
            nc.sync.dma_start(out=wpT_all, in_=wpT_d[:, :].bitcast(f32r))

            # ================= Attention + proj =================
            attn_pools = (
                tc.tile_pool(name="attT", bufs=1),
                tc.tile_pool(name="att", bufs=2),
                tc.tile_pool(name="hop", bufs=2),
                tc.tile_pool(name="outp", bufs=2),
                tc.tile_pool(name="xs", bufs=2),
            )
            attTp = attn_pools[0].__enter__()
            attp = attn_pools[1].__enter__()
            hop = attn_pools[2].__enter__()
            outp = attn_pools[3].__enter__()
            xsp = attn_pools[4].__enter__()
            attT = attTp.tile([P, IT, 512], f32r, name="attT")
            att_tiles = {}

            def emit_scores(it):
                att = attp.tile([P, W], f32r, tag="att", name=f"att{it}")
                att_tiles[it] = att
                srows = gnp.tile([P, NW], f32, tag="srows", name=f"srows{it}")
                for jc in range(NW):
                    ps_s = ps_small.tile([P, 512], f32, tag="ps512",
                                         name=f"sc{it}_{jc}")
                    for ct in range(CT):
                        nc.tensor.matmul(
                            ps_s[:],
                            lhsT=q_sb[ct][:, it * P:(it + 1) * P],
                            rhs=k_sb[ct][:, jc * 512:(jc + 1) * 512],
                            start=(ct == 0), stop=(ct == CT - 1))
                    nc.scalar.activation(out=att[:, jc * 512:(jc + 1) * 512],
                                         in_=ps_s, func=Exp,
                                         bias=0.0, scale=1.0,
                                         accum_out=srows[:, jc:jc + 1])
                srow = gnp.tile([P, 1], f32, tag="srow", name=f"srow{it}")
                nc.vector.reduce_sum(srow, srows, axis=mybir.AxisListType.X)
                rec = gnp.tile([P, 1], f32, tag="rec", name=f"rec{it}")
                nc.vector.reciprocal(rec, srow)
                nc.vector.tensor_scalar_mul(att, att, rec)

            def emit_transposes(it):
                att = att_tiles.pop(it)
                s = it % IGRP
                for jt4 in range(4):
                    ps_t = ps_small.tile([P, 512], f32r, tag="ps512",
                                         name=f"pst{it}_{jt4}")
                    for j4 in range(4):
                        jt = jt4 * 4 + j4
                        nc.tensor.transpose(
                            ps_t[:, j4 * P:(j4 + 1) * P],
                            att[:, jt * P:(jt + 1) * P], ident_r)
                    nc.vector.tensor_copy(
                        out=attT[:, jt4 * 4:jt4 * 4 + 4, s * P:(s + 1) * P],
                        in_=ps_t.rearrange("p (a b) -> p a b", a=4))

            def emit_ho_proj(g):
                ho_sb = []
                for ct in range(CT):
                    ps_ho = ps_small.tile([P, 512], f32, tag="ps512",
                                          name=f"psho{g}_{ct}")
                    for jt in range(IT):
                        nc.tensor.matmul(
                            ps_ho[:],
                            lhsT=vT_sb[jt][:, ct * P:(ct + 1) * P],
                            rhs=attT[:, jt, :],
                            start=(jt == 0), stop=(jt == IT - 1))
                    ho = hop.tile([P, 512], f32r, tag=f"ho{ct}", name=f"ho{g}_{ct}")
                    nc.vector.tensor_copy(out=ho, in_=ps_ho)
                    ho_sb.append(ho)
                for ot in range(CT):
                    ps_o = ps_small.tile([P, 512], f32, tag="ps512",
                                         name=f"pso{g}_{ot}")
                    for ct in range(CT):
                        nc.tensor.matmul(
                            ps_o[:],
                            lhsT=wpT_sb[ct][:, ot * P:(ot + 1) * P],
                            rhs=ho_sb[ct],
                            start=(ct == 0), stop=(ct == CT - 1))
                    xs = xsp.tile([P, 512], f32, tag="xs", name=f"xs{g}_{ot}")
                    nc.sync.dma_start(
                        out=xs,
                        in_=x_d[ot * P:(ot + 1) * P, g * 512:(g + 1) * 512])
                    tmp = outp.tile([P, 512], f32, tag="tmp", name=f"tmp{g}_{ot}")
                    nc.vector.tensor_add(out=tmp, in0=ps_o, in1=xs)
                    osb = outp.tile([P, 512], f32, tag="osb", name=f"osb{g}_{ot}")
                    nc.scalar.activation(out=osb, in_=tmp, func=Ident,
                                         bias=bp_sb[:, ot:ot + 1], scale=1.0)
                    nc.sync.dma_start(
                        out=out_d[ot * P:(ot + 1) * P, g * 512:(g + 1) * 512],
                        in_=osb)

            for step in range(IT + 1):
                if step < IT:
                    emit_scores(step)
                if step >= 1:
                    emit_transposes(step - 1)
                if step % IGRP == 0 and step >= IGRP:
                    emit_ho_proj(step // IGRP - 1)
            for pcm in reversed(attn_pools):
                pcm.__exit__(None, None, None)

    nc.finalize()
    return nc


@functools.lru_cache(maxsize=1)
def _built():
    return _build_nc()


def kernel(x, gn_gamma, gn_beta, wq, bq, wk, bk, wv, bv, wp, bp):
    global LAST_EXEC_NS, LAST_TRACE_PATH
    from concourse.bass_utils import run_bass_kernel_spmd

    x = np.asarray(x, dtype=np.float32)
    scale = float(C) ** -0.5
    f = np.float32
    def pmajor(wT):
        # (C_in, C_out) -> [P, CT*C]: row p holds tiles t=0..CT-1 of wT
        return np.ascontiguousarray(
            wT.reshape(CT, P, C).transpose(1, 0, 2).reshape(P, CT * C))

    wqT = pmajor(np.asarray(wq, f).T * f(scale))
    wkT = pmajor(np.asarray(wk, f).T)
    wvT = pmajor(np.asarray(wv, f).T)
    wpT = pmajor(np.asarray(wp, f).T)
    bq_s = (np.asarray(bq, f) * f(scale)).reshape(C, 1)
    bk_c = np.asarray(bk, f).reshape(C, 1)
    # v bias folds through the (row-stochastic) attention average into proj
    bp_eff = (np.asarray(bp, f) + np.asarray(wp, f) @ np.asarray(bv, f)).reshape(C, 1)
    gam = np.asarray(gn_gamma, f).reshape(C, 1)
    bet = np.asarray(gn_beta, f).reshape(C, 1)

    gsz = C // G
    aux = np.zeros((P, 1044), dtype=f)
    for t in range(CT):
        for p in range(P):
            aux[p, t * P + p // gsz] = 1.0 / gsz          # S selector
            for cl in range(P):
                if p == cl // gsz:
                    aux[p, 512 + t * P + cl] = 1.0        # ST selector
    bqr = bq_s.reshape(CT, P).T
    bkr = bk_c.reshape(CT, P).T
    bpr = bp_eff.reshape(CT, P).T
    aux[:, 1024:1028] = bqr
    aux[:, 1028:1032] = bkr
    aux[:, 1032:1036] = bpr
    aux[:, 1036:1040] = gam.reshape(CT, P).T
    aux[:, 1040:1044] = bet.reshape(CT, P).T

    shared = dict(wqT=wqT, wkT=wkT, wvT=wvT, wpT=wpT, aux=aux)
    in_maps = [dict(x=np.ascontiguousarray(x[i]), **shared) for i in range(B)]

    nc = _built()
    res = run_bass_kernel_spmd(nc, in_maps, list(range(B)), trace=TRACE)
    if TRACE:
        LAST_EXEC_NS = res.exec_time_ns
        if res.instructions_and_trace is not None:
            LAST_TRACE_PATH = res.instructions_and_trace[1]
    return np.stack([res.results[i]["out"] for i in range(B)], axis=0)
